# revision 38
# baseline (speedup 1.0000x reference)
"""Channel-wise row attention kernel for Trainium2 (8 NeuronCores).

Reference computation (per (n, w) slab, with qp = q[n,:,:,w].T etc. of shape (H, C)):
    attn = softmax(qp @ kp.T / sqrt(C), axis=-1);  out_slab = (attn @ vp).T  # (C, H)

Sharding: (n, w-quarter) across 8 cores -> each core owns 64 independent slabs;
no cross-core communication.  All device I/O is bf16 (host casts): per-core DMA
drops 64MB -> 32MB, taking the (exclusive, ~360 B/ns) DMA-engine resource off
the critical path; innermost contiguous runs are kept >= 512B so no half-rate
penalty applies.  Accuracy: bf16 E-matrix rounding (~0.4% per weight) bounds
max rel err at ~6e-3 (fp8 E was tried: 4.4e-2, fails the 2e-2 gate).

Device (per slab, partition-dim softmax -> no on-chip transposes):
    S^T[g,h] = sum_c k[c,g] q[c,h]      4 matmuls into 2x [128,2,512] PSUM
    E = exp(S^T / sqrt(C))              ScalarE, one [128,1024] act per PSUM
                                        tile (amortizes the access bubble)
    fa = e0+e1 (GpSimd), fb = e2+e3,    fold tree split across Pool/DVE
    fc = fa+fb (DVE)                    (GPSIMD may not touch PSUM, SBUF ok)
    colsum = ones^T @ fc                1 matmul (vs 4 unfolded)
    r = 1/colsum                        DVE reciprocal
    O[c,h] = sum_g v^T-block @ E        4 accumulating matmuls
    out = O * r                         DVE multiply, PSUM->SBUF, bf16 out
Head (S+exp+folds) and tail (O+colsum+recip+mul) stages are software-pipelined
2 slabs apart so no in-order SEQ parks on a semaphore ahead of ready work.
Engine busy per core (cost model): Act 134us (bottleneck), PE 126us, DVE
127us, DMA 93us, Pool 72us; makespan ~146us.
PSUM: 2x2 banks (S, double-buffered) + 2 (colsum) + 2 (O) = 8 banks exactly.
"""

import numpy as np
from contextlib import ExitStack

import ml_dtypes

import concourse.bass as bass
import concourse.bacc as bacc
import concourse.tile as tile
import concourse.mybir as mybir
from concourse.bass_utils import run_bass_kernel_spmd

N, C, H, W = 2, 128, 512, 256
NCORES = 8
WQ = 4                 # w-quarters per n
WPC = W // WQ          # 64 slabs per core
GT = H // 128          # 4 g-tiles per slab
import os as _os
G = int(_os.environ.get("KB_G", "2"))  # slabs per DMA group
SCALE = float(1.0 / np.sqrt(np.float32(C)))
F32 = mybir.dt.float32
BF16 = mybir.dt.bfloat16
NP_BF16 = ml_dtypes.bfloat16


def _body(ctx: ExitStack, tc: tile.TileContext, qd, kd, vd, od, n_slabs: int):
    import os

    nc = tc.nc
    s_bufs = int(os.environ.get("KB_S_BUFS", "2"))
    cs_bufs = int(os.environ.get("KB_CS_BUFS", "2"))
    o_bufs = int(os.environ.get("KB_O_BUFS", "2"))
    pipe = int(os.environ.get("KB_PIPE", "2"))  # tail-stage delay in slabs
    const_pool = ctx.enter_context(tc.tile_pool(name="const", bufs=1))
    in_pool = ctx.enter_context(tc.tile_pool(name="inp", bufs=int(os.environ.get("KB_INBUFS", "3"))))
    e_pool = ctx.enter_context(tc.tile_pool(name="epool", bufs=int(os.environ.get("KB_EBUFS", "4"))))
    f_pool = ctx.enter_context(tc.tile_pool(name="fpool", bufs=int(os.environ.get("KB_FBUFS", "3"))))
    r_pool = ctx.enter_context(tc.tile_pool(name="rpool", bufs=int(os.environ.get("KB_RBUFS", "3"))))
    out_pool = ctx.enter_context(tc.tile_pool(name="outp", bufs=2))
    ps_s = ctx.enter_context(tc.tile_pool(name="ps_s", bufs=s_bufs, space="PSUM"))
    ps_cs = ctx.enter_context(tc.tile_pool(name="ps_cs", bufs=cs_bufs, space="PSUM"))
    ps_o = ctx.enter_context(tc.tile_pool(name="ps_o", bufs=o_bufs, space="PSUM"))

    ones_t = const_pool.tile([128, 128], BF16, name="ones_t")
    if os.environ.get("KB_ONESMEMSET", "1") == "1":
        nc.vector.memset(ones_t, 1.0)
    else:
        ones_f32 = const_pool.tile([128, 128], F32, name="ones_f32")
        nc.vector.memset(ones_f32, 1.0)
        nc.scalar.activation(ones_t, ones_f32, mybir.ActivationFunctionType.Copy)
    # PE p-state warmup: ~10 dummy matmuls keep the tensor engine continuously
    # busy while the first input DMAs land, so real matmuls start at full clock
    # (the cost model ramps 0.65->2.4GHz over 3us of continuous execution).
    warm_rhs = const_pool.tile([128, 512], BF16, name="warm_rhs")
    nc.vector.memset(warm_rhs, 0.0)
    n_warm = int(os.environ.get("KB_WARM", "6"))
    for w in range(n_warm):
        wp = ps_cs.tile([128, H], F32, tag="cs", name="warm_ps")
        nc.tensor.matmul(wp, lhsT=ones_t, rhs=warm_rhs, start=True, stop=True)

    # Per-slab state carried between the head stage (S matmuls, exp, folds)
    # and the tail stage (colsum, reciprocal, O matmuls, final multiply),
    # which runs `pipe` slabs behind so no in-order engine queue ever parks
    # on a semaphore while ready work sits behind it.
    state: dict[int, tuple] = {}

    pending_v: list = []

    def load_group(w0: int, gs: int):
        q_g = in_pool.tile([C, gs, H], BF16, tag="q", name="q_g")
        nc.sync.dma_start(out=q_g, in_=qd[w0 : w0 + gs].rearrange("s c h -> c s h"))
        k_g = in_pool.tile([C, gs, H], BF16, tag="k", name="k_g")
        nc.sync.dma_start(out=k_g, in_=kd[w0 : w0 + gs].rearrange("s c h -> c s h"))
        # v is only read by the tail stage (pipe slabs later): defer its load
        # one group so the next group's q/k clear the serialized HWDGE stage
        # (~625ns/DMA) earlier.
        while pending_v:
            pending_v.pop(0)()
        v_g = in_pool.tile([128, gs, GT, C], BF16, tag="v", name="v_g")
        pending_v.append(
            lambda v_g=v_g, w0=w0, gs=gs: nc.sync.dma_start(
                out=v_g, in_=vd[:, w0 : w0 + gs]
            )
        )
        out_g = out_pool.tile([C, gs, H], BF16, tag="out", name="out_g")
        return q_g, k_g, v_g, out_g, w0, gs

    hiprio = int(os.environ.get("KB_HIPRIO", "8"))
    fast0 = os.environ.get("KB_FAST0", "1") == "1"

    def load_group0():
        """Slab-0 fast start: q whole, k in h-halves (the s_a matmuls need
        only k[:, :256]), v deferred -- the first exp launches one k-half
        earlier without flooding the serialized HWDGE stage."""
        HH = H // 2
        q_g = in_pool.tile([C, 1, H], BF16, tag="q", name="q_g")
        nc.sync.dma_start(out=q_g[:, 0, :], in_=qd[0])
        k_g = in_pool.tile([C, 1, H], BF16, tag="k", name="k_g")
        for sl in (slice(0, HH), slice(HH, H)):
            nc.sync.dma_start(out=k_g[:, 0, sl], in_=kd[0, :, sl])
        v_g = in_pool.tile([128, 1, GT, C], BF16, tag="v", name="v_g")
        pending_v.append(
            lambda: nc.sync.dma_start(out=v_g, in_=vd[:, 0:1])
        )
        out_g = out_pool.tile([C, 1, H], BF16, tag="out", name="out_g")
        return q_g, k_g, v_g, out_g, 0, 1

    def head0(grp):
        """Slab-0 head with h-split S matmuls + quarter exps: the first exp
        needs only q[:, :256] and k[:, :256]. Extra activation bubbles land
        in front-idle time the Act engine has anyway."""
        q_g, k_g, v_g, out_g, w0, gs = grp
        q_t = q_g[:, 0, :]
        k_t = k_g[:, 0, :]
        HH = H // 2
        tiles = []
        for tag, toff in (("sa", 0), ("sb", 2)):
            s_x = ps_s.tile([128, 2, H], F32, tag="s", name="s_" + tag)
            e_x = e_pool.tile([128, 2, H], BF16, tag="e", name="e_" + tag)
            for sl in (slice(0, HH), slice(HH, H)):
                for t in (0, 1):
                    nc.tensor.matmul(
                        s_x[:, t, sl],
                        lhsT=k_t[:, (toff + t) * 128 : (toff + t + 1) * 128],
                        rhs=q_t[:, sl],
                        start=True,
                        stop=True,
                    )
                nc.scalar.activation(
                    e_x[:, :, sl], s_x[:, :, sl],
                    mybir.ActivationFunctionType.Exp, scale=SCALE,
                )
            tiles.append(e_x)
        e_a, e_b = tiles

        fa = f_pool.tile([128, H], BF16, tag="fa", name="fa")
        nc.gpsimd.tensor_add(fa, e_a[:, 0, :], e_a[:, 1, :])
        fb = f_pool.tile([128, H], BF16, tag="fb", name="fb")
        nc.vector.tensor_add(fb, e_b[:, 0, :], e_b[:, 1, :])
        fc = f_pool.tile([128, H], BF16, tag="fc", name="fc")
        nc.vector.tensor_add(fc, fa, fb)
        state[len(state) + _done[0]] = (e_a, e_b, fc, grp, 0)

    def head(jj: int, grp):
        q_g, k_g, v_g, out_g, w0, gs = grp
        q_t = q_g[:, jj, :]
        k_t = k_g[:, jj, :]

        from contextlib import nullcontext
        prio = tc.high_priority(offset=hiprio) if hiprio else nullcontext()
        with prio:
            s_a = ps_s.tile([128, 2, H], F32, tag="s", name="s_a")
            for t in (0, 1):
                nc.tensor.matmul(
                    s_a[:, t, :],
                    lhsT=k_t[:, t * 128 : (t + 1) * 128],
                    rhs=q_t,
                    start=True,
                    stop=True,
                )
            e_a = e_pool.tile([128, 2, H], BF16, tag="e", name="e_a")
            nc.scalar.activation(
                e_a, s_a, mybir.ActivationFunctionType.Exp, scale=SCALE
            )

            s_b = ps_s.tile([128, 2, H], F32, tag="s", name="s_b")
            for t in (0, 1):
                nc.tensor.matmul(
                    s_b[:, t, :],
                    lhsT=k_t[:, (t + 2) * 128 : (t + 3) * 128],
                    rhs=q_t,
                    start=True,
                    stop=True,
                )
            e_b = e_pool.tile([128, 2, H], BF16, tag="e", name="e_b")
            nc.scalar.activation(
                e_b, s_b, mybir.ActivationFunctionType.Exp, scale=SCALE
            )

        fa = f_pool.tile([128, H], BF16, tag="fa", name="fa")
        nc.gpsimd.tensor_add(fa, e_a[:, 0, :], e_a[:, 1, :])
        poolf = int(os.environ.get("KB_POOLF", "0"))
        sj = len(state) + _done[0]
        fb_eng = nc.gpsimd if (poolf and (sj % 4) < poolf) else nc.vector
        fb = f_pool.tile([128, H], BF16, tag="fb", name="fb")
        fb_eng.tensor_add(fb, e_b[:, 0, :], e_b[:, 1, :])
        fc = f_pool.tile([128, H], BF16, tag="fc", name="fc")
        nc.vector.tensor_add(fc, fa, fb)

        state[len(state) + _done[0]] = (e_a, e_b, fc, grp, jj)

    def tail(j: int, last: bool = False):
        e_a, e_b, fc, grp, jj = state.pop(j)
        _done[0] += 1
        q_g, k_g, v_g, out_g, w0, gs = grp

        o_ps = ps_o.tile([128, H], F32, tag="o", name="o_ps")
        for t in range(GT):
            e_half = (e_a, e_b)[t // 2]
            nc.tensor.matmul(
                o_ps,
                lhsT=v_g[:, jj, t, :],
                rhs=e_half[:, t % 2, :],
                start=(t == 0),
                stop=(t == GT - 1),
            )
        cs_ps = ps_cs.tile([128, H], F32, tag="cs", name="cs_ps")
        if last:
            # final slab: bypass the DVE fold chain (its latency would sit
            # exposed at the very end) -- accumulate colsum from e directly
            for t in range(GT):
                e_half = (e_a, e_b)[t // 2]
                nc.tensor.matmul(
                    cs_ps, lhsT=ones_t, rhs=e_half[:, t % 2, :],
                    start=(t == 0), stop=(t == GT - 1),
                )
        else:
            nc.tensor.matmul(cs_ps, lhsT=ones_t, rhs=fc, start=True, stop=True)
        from contextlib import nullcontext
        tprio = int(os.environ.get("KB_TPRIO", "0"))
        with (tc.high_priority(offset=tprio) if tprio else nullcontext()):
            r_t = r_pool.tile([128, H], F32, tag="r", name="r_t")
            nc.vector.reciprocal(r_t, cs_ps)
            nc.vector.tensor_mul(out_g[:, jj, :], o_ps, r_t)

        if jj == gs - 1:  # group complete -> store it
            nc.sync.dma_start(
                out=od[w0 : w0 + gs].rearrange("s c h -> c s h"), in_=out_g
            )

    # Group layout: singleton groups at the edges (first S starts after a
    # 1-slab DMA; the final store only waits on the last slab), G-sized in
    # the middle.
    n_edge = int(os.environ.get("KB_EDGE", "3"))
    sizes = [1] * n_edge
    rem = n_slabs - 2 * n_edge
    while rem > 0:
        sizes.append(min(G, rem))
        rem -= min(G, rem)
    sizes += [1] * n_edge
    _done = [0]

    j = 0
    for gs in sizes:
        w0 = j
        if fast0 and w0 == 0:
            grp = load_group0()
            head(0, grp)
            if j >= pipe:
                tail(j - pipe)
            j += 1
            continue
        grp = load_group(w0, gs)
        for jj in range(gs):
            head(jj, grp)
            if j >= pipe:
                tail(j - pipe)
            j += 1
    while pending_v:
        pending_v.pop(0)()
    for j2 in range(n_slabs - pipe, n_slabs):
        tail(j2, last=(j2 == n_slabs - 1))


def build_nc(n_slabs: int = WPC) -> bass.Bass:
    nc = bacc.Bacc("TRN2", target_bir_lowering=False, debug=False)
    qd = nc.dram_tensor("q", [n_slabs, C, H], BF16, kind="ExternalInput").ap()
    kd = nc.dram_tensor("k", [n_slabs, C, H], BF16, kind="ExternalInput").ap()
    vd = nc.dram_tensor("v", [128, n_slabs, GT, C], BF16, kind="ExternalInput").ap()
    od = nc.dram_tensor("o", [n_slabs, C, H], BF16, kind="ExternalOutput").ap()
    with tile.TileContext(nc) as tc, ExitStack() as ctx:
        _body(ctx, tc, qd, kd, vd, od, n_slabs)
    nc.compile()
    return nc


def shard_inputs(q: np.ndarray, k: np.ndarray, v: np.ndarray) -> list[dict]:
    """Host-side shard + permute: core i gets n = i // WQ, w in [64*(i%WQ), ...).
    q/k -> (W', C, H) bf16; v -> (p, W', t, c) bf16 with H = t*128 + p."""
    in_maps = []
    for i in range(NCORES):
        n, wq = divmod(i, WQ)
        ws = slice(wq * WPC, (wq + 1) * WPC)
        qs = np.ascontiguousarray(
            np.transpose(q[n, :, :, ws], (2, 0, 1)).astype(NP_BF16)
        )
        ks = np.ascontiguousarray(
            np.transpose(k[n, :, :, ws], (2, 0, 1)).astype(NP_BF16)
        )
        # v[n]: (C, H, W') -> view H as (t, p) -> (p, W', t, c)
        vs = np.ascontiguousarray(
            np.transpose(
                v[n, :, :, ws].reshape(C, GT, 128, WPC), (2, 3, 1, 0)
            ).astype(NP_BF16)
        )
        in_maps.append({"q": qs, "k": ks, "v": vs})
    return in_maps


def unshard_output(results: list[dict]) -> np.ndarray:
    out = np.empty((N, C, H, W), dtype=np.float32)
    for i in range(NCORES):
        n, wq = divmod(i, WQ)
        ws = slice(wq * WPC, (wq + 1) * WPC)
        # (W', C, H) -> (C, H, W')
        out[n, :, :, ws] = np.transpose(
            np.asarray(results[i]["o"]).astype(np.float32), (1, 2, 0)
        )
    return out


_NC_CACHE = {}


def kernel(q: np.ndarray, k: np.ndarray, v: np.ndarray, **run_kwargs) -> np.ndarray:
    q = np.asarray(q, dtype=np.float32)
    k = np.asarray(k, dtype=np.float32)
    v = np.asarray(v, dtype=np.float32)
    if "default" not in _NC_CACHE:
        _NC_CACHE["default"] = build_nc()
    nc = _NC_CACHE["default"]
    in_maps = shard_inputs(q, k, v)
    res = run_bass_kernel_spmd(nc, in_maps, core_ids=list(range(NCORES)), **run_kwargs)
    out = unshard_output(res.results)
    if run_kwargs.get("trace"):
        kernel.last_result = res
    return out

